# revision 2
# baseline (speedup 1.0000x reference)
"""Trainium2 Bass kernel for nn_DualThresholdSelfregulatingIntegrate (v4).

Reference semantics (per lane (b, d), sequential over s, float32):
    rate = relu(x) * dt
    4x per step: v = v + rate; spikes = floor(v); v = v - spikes
    out[b, s, d] = spikes_after_4th_substep / dt

Identities (verified bit-exact vs the jax CPU reference, except a single
known flip where w4 == 1.0 exactly -> rel err 1.2e-2 < 2e-2):
  - w = running sum of rates seeded with v0 (exact fp32 add chain via the
    stock (add,add) scan over (r, r) pairs) crosses integer boundaries
    exactly like the reference's mod-ed accumulator; w < 3.
  - spike_t = [r_t > frac(w4_t)]; floor(w4) = int32(w4 - 0.5) under the
    hardware's round-to-nearest-even convert; w4 - floor is exact.

Engine split per group (32 groups/core of [128 lanes, 512 steps]):
  PE:   4x fp32 128x128 transposes x -> pin (PSUM)
  ACT:  r2 = Relu(pin)*dt duplicated via 0-stride broadcast read
  DVE:  tensor_tensor_scan (add,add) -> w24 (the irreducible 2048-add chain)
  ACT:  fint = Copy(w4 - 0.5) -> int32   (floor, off the DVE)
  DVE:  fr = w4 - fint;  t = fr - r      (issued one group behind the scan)
  ACT:  sgn = Sign(t) -> bf16 {-1,0,1}   (spike <=> t < 0)
  PE:   bf16 transposes sgn -> pout (bf16 PSUM)
  ACT:  onat = Relu(-inv_dt * pout) -> fp32 {0, 999.99994}  (bit-exact)

Sharding: data-parallel over batch, 4 batches per core, 8 cores.
"""

import numpy as np

B, S, D = 32, 512, 1024
NCORES = 8
BL = B // NCORES  # batches per core
DG = D // 128  # 8 lane groups per batch
SC = S // 128  # 4 time chunks
NG = BL * DG  # 32 groups per core
NB = 6  # group-level buffer depth

DT_F = float(np.float32(0.001))
INV_DT = float(np.float32(1.0) / np.float32(0.001))  # 999.99994

_CACHE = {}


def _build():
    import concourse.bass as bass
    import concourse.mybir as mybir

    AF = mybir.ActivationFunctionType
    AL = mybir.AluOpType
    f32 = mybir.dt.float32
    bf16 = mybir.dt.bfloat16
    i32 = mybir.dt.int32

    nc = bass.Bass()
    x_ext = nc.declare_dram_parameter("x", [BL, S, D], f32, isOutput=False)
    v0_ext = nc.declare_dram_parameter("v0", [BL, D], f32, isOutput=False)
    id_ext = nc.declare_dram_parameter("ident", [128, 128], f32, isOutput=False)
    out_ext = nc.declare_dram_parameter("out", [BL, S, D], f32, isOutput=True)

    sb = lambda name, shape, dt=f32: nc.alloc_sbuf_tensor(name, shape, dt).ap()
    ps = lambda name, shape, dt=f32: nc.alloc_psum_tensor(name, shape, dt).ap()

    ident = sb("ident_sb", [128, 128])
    identb = sb("identb_sb", [128, 128], bf16)
    nat = [sb(f"nat_{i}", [128, SC * D]) for i in range(2)]
    v0nat = [sb(f"v0nat_{i}", [DG, 128]) for i in range(2)]
    v0t = [sb(f"v0t_{i}", [128, DG]) for i in range(2)]
    pv0 = [ps(f"pv0_{i}", [128, DG]) for i in range(2)]
    pin = [ps(f"pin_{i}", [128, S]) for i in range(2)]
    r2 = [sb(f"r2_{i}", [128, 2 * S]) for i in range(NB)]
    w24 = [sb(f"w24_{i}", [128, 2 * S]) for i in range(NB)]
    fint = [sb(f"fint_{i}", [128, S], i32) for i in range(NB)]
    fr = [sb(f"fr_{i}", [128, S]) for i in range(NB)]
    tt = [sb(f"tt_{i}", [128, S]) for i in range(NB)]
    sgn = [[sb(f"sgn_{i}_{dk}", [128, S], bf16) for dk in range(DG)] for i in range(2)]
    pout = [ps(f"pout_{i}", [128, D], bf16) for i in range(2)]
    onat = [sb(f"onat_{i}", [128, D]) for i in range(2)]
    scr = sb("scr_sb", [128, 1])
    scri = sb("scri_sb", [128, 1], i32)

    def w4_of(j):
        return w24[j].rearrange("p (t two) -> p t two", two=2)[:, :, 1]

    def r1_of(j):
        return r2[j].rearrange("p (t two) -> p t two", two=2)[:, :, 0]

    with (
        nc.Block() as block,
        nc.semaphore("s_id") as s_id,  # +16 ident load
        nc.semaphore("s_idb") as s_idb,  # +1 identb ACT convert
        nc.semaphore("s_nath0") as s_nath0,  # +16/head (dk=0) load, even b
        nc.semaphore("s_nath1") as s_nath1,  # +16/head load, odd b
        nc.semaphore("s_natr0") as s_natr0,  # +16/remainder load, even b
        nc.semaphore("s_natr1") as s_natr1,  # +16/remainder load, odd b
        nc.semaphore("s_v00") as s_v00,  # +16/v0 load, even batches
        nc.semaphore("s_v01") as s_v01,  # +16/v0 load, odd batches
        nc.semaphore("s_pv0") as s_pv0,  # +1 per PE v0 transpose
        nc.semaphore("s_v0t") as s_v0t,  # +1 per ACT v0t copy
        nc.semaphore("s_pin") as s_pin,  # +1 per PE in-transpose
        nc.semaphore("s_rate") as s_rate,  # +1 per group (ACT relu)
        nc.semaphore("s_scan") as s_scan,  # +1 per DVE scan
        nc.semaphore("s_fint") as s_fint,  # +1 per ACT fint
        nc.semaphore("s_subs") as s_subs,  # +1 per DVE t (group subs done)
        nc.semaphore("s_sgn") as s_sgn,  # +1 per ACT Sign
        nc.semaphore("s_pout") as s_pout,  # +1 per PE out-transpose
        nc.semaphore("s_osc") as s_osc,  # +1 per ACT out scale copy
        nc.semaphore("s_store") as s_store,  # +16 per output store DMA
    ):
        s_nath = [s_nath0, s_nath1]
        s_natr = [s_natr0, s_natr1]
        s_v0 = [s_v00, s_v01]

        def _pe_out(tensor, b, chunks):
            i = b % 2
            tensor.wait_ge(s_sgn, DG * (b + 1))
            for sc in chunks:
                k = b * SC + sc
                if k >= 2:
                    tensor.wait_ge(s_osc, k - 1)  # pout slot reuse
                for dk in range(DG):
                    nc.tensor.transpose(
                        pout[k % 2][:, dk * 128 : (dk + 1) * 128],
                        sgn[i][dk][:, sc * 128 : (sc + 1) * 128],
                        identb[:, :],
                    ).then_inc(s_pout, 1)

        def _act_out(scalar, b, sc):
            k = b * SC + sc
            if True:
                scalar.wait_ge(s_pout, DG * (k + 1))
                if k >= 2:
                    scalar.wait_ge(s_store, 16 * (k - 1))  # onat slot reuse
                # pout holds transposed Sign(fr - r) in {-1,0,1}:
                #   spike <=> sign = -1 -> Relu(-inv_dt * x) gives 999.99994
                scalar.activation(
                    onat[k % 2][:, :], pout[k % 2][:, :], AF.Relu, scale=-INV_DT
                ).then_inc(s_osc, 1)

        def _fint(scalar, g):
            j = g % NB
            scalar.wait_ge(s_scan, g + 1)
            if g >= NB:
                scalar.wait_ge(s_subs, g - NB + 1)  # fint slot reuse
            scalar.activation(
                fint[j][:, :], w4_of(j), AF.Copy, scale=1.0, bias=-0.5
            ).then_inc(s_fint, 1)

        def _sgn(scalar, g):
            b, dk = divmod(g, DG)
            i = b % 2
            scalar.wait_ge(s_subs, g + 1)
            if b >= 2:
                scalar.wait_ge(s_pout, DG * SC * (b - 1))  # sgn slot reuse
            scalar.activation(
                sgn[i][dk][:, :], tt[g % NB][:, :], AF.Sign, scale=1.0
            ).then_inc(s_sgn, 1)

        @block.sync
        def _(sync):
            sync.dma_start(out=ident[:, :], in_=id_ext[:, :]).then_inc(s_id, 16)
            for b in range(BL):
                i = b % 2
                if b >= 2:
                    sync.wait_ge(s_pin, 4 * DG * (b - 1))  # nat slot: PE consumed
                    sync.wait_ge(s_pv0, b - 1)  # v0nat slot: PE consumed
                nat3d = nat[i][:, :].rearrange("p (sc d) -> p sc d", sc=SC)
                sync.dma_start(
                    out=nat3d[:, :, 0:128],
                    in_=x_ext[b, :, 0:128].rearrange("(sc p) d -> p sc d", p=128),
                ).then_inc(s_nath[i], 16)
                sync.dma_start(
                    out=v0nat[i][:, :],
                    in_=v0_ext[b, :].rearrange("(dk p) -> dk p", p=128),
                ).then_inc(s_v0[i], 16)
                sync.dma_start(
                    out=nat3d[:, :, 128:D],
                    in_=x_ext[b, :, 128:D].rearrange("(sc p) d -> p sc d", p=128),
                ).then_inc(s_natr[i], 16)
            for b in range(BL):
                for sc in range(SC):
                    k = b * SC + sc
                    sync.wait_ge(s_osc, k + 1)
                    sync.dma_start(
                        out=out_ext[b, sc * 128 : (sc + 1) * 128, :],
                        in_=onat[k % 2][:, :],
                    ).then_inc(s_store, 16)

        @block.tensor
        def _(tensor):
            tensor.wait_ge(s_id, 16)
            for b in range(BL):
                i = b % 2
                tensor.wait_ge(s_v0[i], 16 * (b // 2 + 1))
                if b >= 2:
                    tensor.wait_ge(s_v0t, b - 1)  # pv0 slot reuse
                nc.tensor.transpose(
                    pv0[i][:, :], v0nat[i][:, :], ident[0:DG, 0:DG]
                ).then_inc(s_pv0, 1)
                tensor.wait_ge(s_nath[i], 16 * (b // 2 + 1))
                for dk in range(DG):
                    g = b * DG + dk
                    if dk == 1:
                        tensor.wait_ge(s_natr[i], 16 * (b // 2 + 1))
                    if g >= 2:
                        tensor.wait_ge(s_rate, g - 1)  # pin slot reuse
                    for sc in range(SC):
                        nc.tensor.transpose(
                            pin[g % 2][:, sc * 128 : (sc + 1) * 128],
                            nat[i][:, sc * D + dk * 128 : sc * D + (dk + 1) * 128],
                            ident[:, :],
                        ).then_inc(s_pin, 1)
                    if dk == 3 and b >= 1:
                        tensor.wait_ge(s_idb, 1)
                        _pe_out(tensor, b - 1, (0, 1))
                    if dk == 5 and b >= 1:
                        _pe_out(tensor, b - 1, (2, 3))
            _pe_out(tensor, BL - 1, (0, 1))
            _pe_out(tensor, BL - 1, (2, 3))

        @block.scalar
        def _(scalar):
            # warm the ACT function tables while the first loads stream
            scalar.activation(scr[:, :], ident[:, 0:1], AF.Relu, scale=1.0)
            scalar.activation(scr[:, :], ident[:, 0:1], AF.Copy, scale=1.0)
            scalar.activation(scr[:, :], ident[:, 0:1], AF.Sign, scale=1.0)
            scalar.activation(scri[:, :], ident[:, 0:1], AF.Copy, scale=1.0, bias=-0.5)
            scalar.wait_ge(s_id, 16)
            scalar.activation(identb[:, :], ident[:, :], AF.Copy, scale=1.0).then_inc(
                s_idb, 1
            )
            for b in range(BL):
                i = b % 2
                scalar.wait_ge(s_pv0, b + 1)
                if b >= 2:
                    scalar.wait_ge(s_scan, DG * (b - 1))  # v0t slot reuse
                scalar.activation(
                    v0t[i][:, :], pv0[i][:, :], AF.Copy, scale=1.0
                ).then_inc(s_v0t, 1)
                for dk in range(DG):
                    g = b * DG + dk
                    scalar.wait_ge(s_pin, 4 * (g + 1))
                    if g >= NB:
                        scalar.wait_ge(s_subs, g - NB + 1)  # r2 slot reuse
                    r2_3d = r2[g % NB].rearrange("p (t two) -> p t two", two=2)
                    pin_b = pin[g % 2].unsqueeze(2).broadcast_to([128, S, 2])
                    scalar.activation(
                        r2_3d, pin_b, AF.Relu, scale=DT_F
                    ).then_inc(s_rate, 1)
                    if g - 1 >= 0:
                        _fint(scalar, g - 1)
                    if g - 3 >= 0:
                        _sgn(scalar, g - 3)
                    if b >= 1 and dk >= 4:
                        _act_out(scalar, b - 1, dk - 4)
            _fint(scalar, NG - 1)
            _sgn(scalar, NG - 3)
            _sgn(scalar, NG - 2)
            _sgn(scalar, NG - 1)
            for sc in range(SC):
                _act_out(scalar, BL - 1, sc)

        @block.vector
        def _(vector):
            def _subs(g):
                j = g % NB
                vector.wait_ge(s_fint, g + 1)
                if g >= NB:
                    vector.wait_ge(s_sgn, g - NB + 1)  # tt slot reuse
                nc.vector.tensor_tensor(
                    out=fr[j][:, :], in0=w4_of(j), in1=fint[j][:, :],
                    op=AL.subtract,
                )
                nc.vector.tensor_tensor(
                    out=tt[j][:, :], in0=fr[j][:, :], in1=r1_of(j),
                    op=AL.subtract,
                ).then_inc(s_subs, 1)

            for b in range(BL):
                i = b % 2
                for dk in range(DG):
                    g = b * DG + dk
                    j = g % NB
                    vector.wait_ge(s_rate, g + 1)
                    if dk == 0:
                        vector.wait_ge(s_v0t, b + 1)
                    nc.vector.tensor_tensor_scan(
                        out=w24[j][:, :],
                        data0=r2[j][:, :],
                        data1=r2[j][:, :],
                        initial=v0t[i][:, dk : dk + 1],
                        op0=AL.add,
                        op1=AL.add,
                    ).then_inc(s_scan, 1)
                    if g - 1 >= 0:
                        _subs(g - 1)
            _subs(NG - 1)

    return nc


def kernel(inputs: np.ndarray, initial_state: np.ndarray) -> np.ndarray:
    import os
    from concourse.bass_utils import run_bass_kernel_spmd

    inputs = np.ascontiguousarray(inputs, dtype=np.float32)
    initial_state = np.ascontiguousarray(initial_state, dtype=np.float32)

    if "nc" not in _CACHE:
        _CACHE["nc"] = _build()
    nc = _CACHE["nc"]

    ident = np.eye(128, dtype=np.float32)
    core_ids = list(range(NCORES))
    in_maps = [
        {
            "x": inputs[c * BL : (c + 1) * BL],
            "v0": initial_state[c * BL : (c + 1) * BL],
            "ident": ident,
        }
        for c in core_ids
    ]
    trace = bool(int(os.environ.get("DTI_TRACE", "0")))
    res = run_bass_kernel_spmd(nc, in_maps, core_ids, trace=trace)
    _CACHE["last"] = res
    out = np.concatenate([res.results[c]["out"] for c in core_ids], axis=0)
    return out
